# revision 14
# baseline (speedup 1.0000x reference)
"""AMMLinear (vq_codebook) forward kernel for 8 TRN2 NeuronCores.

Key algebraic fact: the reference's straight-through estimator
    output = real - stop_grad(real - quantized)
is numerically exactly `quantized_output + bias`, so the forward value needs
only:  argmin-distance one-hot  @  fake-quantized lut  + bias.

Distribution: pure data-parallel over the 8192 tokens (1024/core), zero
collectives.  The quantized lut q = clip(round(lut/scale), -127, 127) is
x-independent, computed EXACTLY on host (float64, matching the oracle) and
shipped as two e4m3-exact fp8 planes q = qa16 + qb (qa16 = 16*round(q/16),
qb = q - qa16); the 0/1 one-hots are fp8-exact too.

Gather: token-major fp8 DoubleRow matmuls, psum[tok128, 1024] accumulating
(4 group-pairs) x (a,b) passes with the one-hot pair stationary (reused
across 4 matmuls each).  The PE moving port is ~1KB/partition/213ns, so
this runs at fp16's column rate (exact int8 = 2x fp8 information) with
half the weight loads.  The psum holds exact integer sums (|.|<=8128), so
the drain is a single fp32->fp16 convert-copy on the otherwise-idle Act
engine, and out DMA is fp16 alternating the sync/scalar queues.  The
x-independent  out * scale + bias  runs on host in fp32.

Scores: fp16 hi/lo 3-pass (xh.bh + xh.bl + xl.bh + fp16 c2 hi/lo K=2 init;
residual ~2^-22 -- measured cheaper than fp32 matmuls, which lower to two
~237ns half-speed passes each).  Full-tile [128,1024] score psum; the DVE
argmax chain (reduce / is_equal / first-hit encode / reduce; each DVE op
has ~500ns fixed overhead so fewer-larger ops win) frees the bank after
is_equal.  The idxt -> idxT transpose uses the DMA XBAR (dma_start_
transpose) instead of a PE transpose -- no PSUM bank, no PE slot, no DVE
copy.  One-hot expansion batches all 8 groups of a token-half into a
single [128,8,512] is_equal.

DMA need-order matters: all queues share ~350GB/s per core, so consts
stream first, then xh tiles, xl tiles, and only then the 8.4MB fp8 lut
(first needed ~20us in), with output DMAs trailing the gather.
"""

import numpy as np

N_TOKENS = 8192
IN_FEAT = 1024
C = 64   # codebooks
KC = 16  # centroids per codebook
S = 16   # subvector length
O = 4096  # out features
NCORES = 8
NLOC = N_TOKENS // NCORES  # 1024 tokens per core
G = 8    # groups of 8 codebooks -> 128-row contraction
GP = 4   # group-pairs (DoubleRow: 2 groups = 256-row contraction)
TT = NLOC // 128  # 8 token tiles
NQ = 4   # o-quarters of 1024 cols (one gather unit each)

_CACHED = {}


def build_nc():
    import concourse.bacc as bacc
    import concourse.mybir as mybir
    import concourse.tile as tile
    from contextlib import ExitStack

    f32 = mybir.dt.float32
    f16 = mybir.dt.float16
    f8 = mybir.dt.float8e4
    AO = mybir.AluOpType
    AF = mybir.ActivationFunctionType
    DR = mybir.MatmulPerfMode.DoubleRow
    X = mybir.AxisListType.X

    nc = bacc.Bacc(
        "TRN2", target_bir_lowering=False, debug=False, num_devices=NCORES
    )

    xh_d = nc.dram_tensor("xh", [128, TT, G, 128], f16, kind="ExternalInput")
    xl_d = nc.dram_tensor("xl", [128, TT, G, 128], f16, kind="ExternalInput")
    q8a_d = nc.dram_tensor("q8a", [128, NQ, G, 1024], f8, kind="ExternalInput")
    q8b_d = nc.dram_tensor("q8b", [128, NQ, G, 1024], f8, kind="ExternalInput")
    bdh_d = nc.dram_tensor("bdh", [128, G, 128], f16, kind="ExternalInput")
    bdl_d = nc.dram_tensor("bdl", [128, G, 128], f16, kind="ExternalInput")
    nc2hl_d = nc.dram_tensor("nc2hl", [2, 1024], f16, kind="ExternalInput")
    or2_d = nc.dram_tensor("or2", [2, 128], f16, kind="ExternalInput")
    kiota_d = nc.dram_tensor("kiota", [128, 1], f16, kind="ExternalInput")
    ioneg_d = nc.dram_tensor("ioneg", [128, 1024], f16, kind="ExternalInput")
    out_d = nc.dram_tensor("out", [NLOC, O], f16, kind="ExternalOutput")

    with ExitStack() as ctx:
        tc = ctx.enter_context(tile.TileContext(nc))
        sb = ctx.enter_context(tc.tile_pool(name="sb", bufs=1))
        sbx = ctx.enter_context(tc.tile_pool(name="sbx", bufs=8))
        sbm = ctx.enter_context(tc.tile_pool(name="sbm", bufs=2))
        sbo = ctx.enter_context(tc.tile_pool(name="sbo", bufs=8))
        sbi = ctx.enter_context(tc.tile_pool(name="sbi", bufs=3))
        psA = ctx.enter_context(tc.tile_pool(name="psA", bufs=2, space="PSUM"))
        psB = ctx.enter_context(tc.tile_pool(name="psB", bufs=2, space="PSUM"))

        # ---------- persistent SBUF ----------
        bdh_sb = sb.tile([128, G, 128], f16)
        bdl_sb = sb.tile([128, G, 128], f16)
        nc2hl_sb = sb.tile([2, 1024], f16)
        or2_sb = sb.tile([2, 128], f16)
        kiota_sb = sb.tile([128, 1], f16)
        ioneg_sb = sb.tile([128, 1024], f16)
        q8a_sb = sb.tile([128, NQ, G, 1024], f8)
        q8b_sb = sb.tile([128, NQ, G, 1024], f8)
        oh8_sb = sb.tile([128, G, NLOC], f8)
        # full-height transpose target (XBAR needs 128-col-multiple input;
        # rows 64..127 are padding and never read)
        idxT_sb = sb.tile([128, NLOC], f16)

        # ---------- const + input DMAs (global need-order) ----------
        nc.gpsimd.dma_start(or2_sb[:], or2_d[:])
        nc.gpsimd.dma_start(nc2hl_sb[:], nc2hl_d[:])
        nc.gpsimd.dma_start(bdh_sb[:], bdh_d[:])
        nc.gpsimd.dma_start(bdl_sb[:], bdl_d[:])
        nc.gpsimd.dma_start(kiota_sb[:], kiota_d[:])
        nc.gpsimd.dma_start(ioneg_sb[:], ioneg_d[:])

        xh_tiles = [
            sbx.tile([128, G, 128], f16, tag="xh", name=f"xh{t}")
            for t in range(TT)
        ]
        xl_tiles = [
            sbx.tile([128, G, 128], f16, tag="xl", name=f"xl{t}")
            for t in range(TT)
        ]
        for t in range(TT):
            nc.scalar.dma_start(xh_tiles[t][:], xh_d[:, t])
        for t in range(TT):
            nc.sync.dma_start(xl_tiles[t][:], xl_d[:, t])
        for qtr in range(NQ):
            nc.sync.dma_start(q8a_sb[:, qtr], q8a_d[:, qtr])
            nc.sync.dma_start(q8b_sb[:, qtr], q8b_d[:, qtr])

        # ---------- phase S: scores -> first-max index encoding ----------
        idxt_tiles = {}
        sc_tiles = {}

        def emit_score_hi(t):
            sc_ps = psA.tile([128, 1024], f32, tag="sc", name=f"sc{t}")
            sc_tiles[t] = sc_ps
            for h in range(2):
                nc.tensor.matmul(
                    sc_ps[:, h * 512 : (h + 1) * 512], or2_sb[:],
                    nc2hl_sb[:, h * 512 : (h + 1) * 512],
                    start=True, stop=False, skip_group_check=True,
                )
            for g in range(G):
                nc.tensor.matmul(
                    sc_ps[:, g * 128 : (g + 1) * 128],
                    xh_tiles[t][:, g, :], bdh_sb[:, g, :],
                    start=False, stop=False, skip_group_check=True,
                )
                nc.tensor.matmul(
                    sc_ps[:, g * 128 : (g + 1) * 128],
                    xh_tiles[t][:, g, :], bdl_sb[:, g, :],
                    start=False, stop=False, skip_group_check=True,
                )

        def emit_score_lo(t):
            sc_ps = sc_tiles[t]
            for g in range(G):
                nc.tensor.matmul(
                    sc_ps[:, g * 128 : (g + 1) * 128],
                    xl_tiles[t][:, g, :], bdh_sb[:, g, :],
                    start=False, stop=(g % 4 == 3), skip_group_check=True,
                )
            # argmax chain on DVE; bank freed after is_equal (2nd op)
            maxb = sbm.tile([128, C], f32, tag="maxb", name=f"maxb{t}")
            nc.vector.tensor_reduce(
                maxb[:], sc_ps[:].rearrange("p (c k) -> p c k", k=KC),
                axis=X, op=AO.max,
            )
            mask = sbm.tile([128, 1024], f16, tag="mask", name=f"mask{t}")
            nc.vector.tensor_tensor(
                mask[:].rearrange("p (c k) -> p c k", k=KC),
                sc_ps[:].rearrange("p (c k) -> p c k", k=KC),
                maxb[:].rearrange("p (c u) -> p c u", u=1).broadcast_to((128, C, KC)),
                op=AO.is_equal,
            )
            # iv = mask*64 + (15-k): max picks the first (smallest-k) hit
            nc.vector.scalar_tensor_tensor(
                mask[:], mask[:], 64.0, ioneg_sb[:], op0=AO.mult, op1=AO.add
            )
            idxt = sbi.tile([128, 128], f16, tag="idxt", name=f"idxt{t}")
            nc.gpsimd.memset(idxt[:, C:128], 0.0)
            nc.vector.tensor_reduce(
                idxt[:, 0:C], mask[:].rearrange("p (c k) -> p c k", k=KC),
                axis=X, op=AO.max,
            )
            idxt_tiles[t] = idxt
            # idxt [128,128] -> idxT[:, tok] via the DMA transpose XBAR (only
            # SP/Act queues have it; Act is idle here): no PE slot, no PSUM
            # bank, no DVE copy.  Rows C..127 of the result are padding.
            nc.scalar.dma_start_transpose(
                idxT_sb[:, t * 128 : (t + 1) * 128], idxt[:]
            )

        # one-hot expansion for token half h: 8 broadcast DMAs fill a
        # [128, 8, 512] index tile, then ONE is_equal writes all 8 groups
        def emit_oh(h):
            cols = slice(h * 512, (h + 1) * 512)
            idxb = sbi.tile([128, G, 512], f16, tag="idxb", name=f"idxb{h}")
            for g in range(G):
                nc.gpsimd.dma_start(
                    idxb[:, g, :],
                    idxT_sb[g * 8 : (g + 1) * 8, cols]
                    .rearrange("j (n u) -> j u n", u=1)
                    .broadcast_to((8, KC, 512)),
                )
            nc.vector.tensor_tensor(
                oh8_sb[:, :, cols], idxb[:],
                kiota_sb[:, 0:1].broadcast_to((128, G, 512)),
                op=AO.is_equal,
            )

        # ---------- phase G: token-major DoubleRow gather units ----------
        def emit_gunit(t, qtr, u):
            tok = slice(t * 128, (t + 1) * 128)
            ps = psB.tile([128, 1024], f32, tag="gu", name=f"gu{t}_{qtr}")
            for gp in range(GP):
                lhs = oh8_sb[:, 2 * gp : 2 * gp + 2, tok]
                for ch in range(2):
                    oc = slice(ch * 512, (ch + 1) * 512)
                    nc.tensor.matmul(
                        ps[:, oc], lhs, q8a_sb[:, qtr, 2 * gp : 2 * gp + 2, oc],
                        start=(gp == 0), stop=False,
                        perf_mode=DR, skip_group_check=True,
                    )
                    nc.tensor.matmul(
                        ps[:, oc], lhs, q8b_sb[:, qtr, 2 * gp : 2 * gp + 2, oc],
                        start=False, stop=(gp == GP - 1),
                        perf_mode=DR, skip_group_check=True,
                    )
            o16 = sbo.tile([128, 1024], f16, tag="o16", name=f"o16_{t}_{qtr}")
            # drains on the otherwise-idle Act engine (GpSimd can't read
            # PSUM); the second-to-last goes to DVE so the tail overlaps
            if u == 30:
                nc.vector.tensor_copy(o16[:], ps[:])
            else:
                nc.scalar.activation(o16[:], ps[:], AF.Identity, bias=0.0, scale=1.0)
            eng = nc.sync if u % 2 == 0 else nc.scalar
            eng.dma_start(out_d[tok, qtr * 1024 : (qtr + 1) * 1024], o16[:])

        # ---------- interleaved emission (PE + DVE queues in-order) ----------
        emit_score_hi(0); emit_score_hi(1)
        emit_score_lo(0)
        emit_score_hi(2)
        emit_score_lo(1)
        emit_score_hi(3)
        emit_score_lo(2)
        emit_score_hi(4)
        emit_score_lo(3)
        emit_oh(0)
        emit_score_hi(5)
        emit_score_lo(4)
        emit_score_hi(6)
        emit_score_lo(5)
        emit_gunit(0, 0, 0)
        emit_score_hi(7)
        emit_score_lo(6)
        emit_gunit(0, 1, 1)
        emit_score_lo(7)
        emit_oh(1)
        u = 2
        for qtr in range(2, NQ):
            emit_gunit(0, qtr, u)
            u += 1
        for t in range(1, TT):
            for qtr in range(NQ):
                emit_gunit(t, qtr, u)
                u += 1

    nc.compile()
    return nc


def _consts():
    kiota = (79.0 - np.arange(128, dtype=np.float32) % KC).reshape(128, 1).astype(np.float16)
    ioneg = np.tile(
        15.0 - (np.arange(1024, dtype=np.float32) % KC), (128, 1)
    ).astype(np.float16)
    return kiota, ioneg


def _prep_inputs(x, centroids, weight, bias):
    """Host-side shard/layout prep, exact int8 fake-quant lut, fp8 split."""
    import ml_dtypes

    kiota, ioneg = _consts()
    # block-diagonal centroids^T: bd[16j+s, g, 16j+k] = centroids[8g+j, k, s]
    bd = np.zeros((128, G, 128), np.float32)
    for g in range(G):
        for j in range(8):
            bd[16 * j : 16 * (j + 1), g, 16 * j : 16 * (j + 1)] = centroids[
                8 * g + j
            ].T
    bdh = bd.astype(np.float16)
    bdl = (bd - bdh.astype(np.float32)).astype(np.float16)
    nc2 = (-0.5 * (centroids.astype(np.float64) ** 2).sum(-1)).astype(
        np.float32
    ).reshape(1, C * KC)
    nc2h = nc2.astype(np.float16)
    nc2l = (nc2 - nc2h.astype(np.float32)).astype(np.float16)
    nc2hl = np.concatenate([nc2h, nc2l], axis=0)
    or2 = np.ones((2, 128), np.float16)

    # exact int8 fake-quant lut (float64, matching the oracle) + fp8 split
    lut = np.einsum(
        "cks,cso->cko", centroids.astype(np.float64), weight.astype(np.float64)
    )
    amax = np.abs(lut).max()
    scale = np.float32(amax / 127.0)
    q = np.clip(np.round(lut / (amax / 127.0)), -127.0, 127.0)
    qa16 = 16.0 * np.round(q / 16.0)   # multiples of 16, |.| <= 128
    qb = q - qa16                      # ints, |.| <= 8

    def pack(v):
        v = v.reshape(G, 8, KC, O)       # [g, j, k, o]
        v = v.transpose(1, 2, 0, 3)      # [j, k, g, o] -> p = 16j+k
        v = v.reshape(128, G, NQ, 1024)  # [p, g, qtr, oc]
        v = v.transpose(0, 2, 1, 3)      # [p, qtr, g, oc]
        return np.ascontiguousarray(v).astype(ml_dtypes.float8_e4m3)

    common = dict(
        q8a=pack(qa16), q8b=pack(qb), bdh=bdh, bdl=bdl, nc2hl=nc2hl,
        or2=or2, kiota=kiota, ioneg=ioneg,
    )
    in_maps = []
    for i in range(NCORES):
        xs = x[i * NLOC : (i + 1) * NLOC, :]  # (1024, 1024)
        xt = np.ascontiguousarray(
            xs.T.reshape(G, 128, TT, 128).transpose(1, 2, 0, 3)
        )  # [p, t, g, n]
        xh = xt.astype(np.float16)
        xl = (xt - xh.astype(np.float32)).astype(np.float16)
        m = dict(common)
        m.update(xh=xh, xl=xl)
        in_maps.append(m)
    return in_maps, scale, np.asarray(bias, np.float32)


def _assemble(per_core_outs, scale, bias32):
    out = np.concatenate(
        [np.asarray(o).astype(np.float32) for o in per_core_outs], axis=0
    )
    return out * scale + bias32[None, :]


def kernel(x, centroids, weight, inverse_temperature_logit, bias, **_):
    from concourse.bass_utils import run_bass_kernel_spmd

    x = np.asarray(x, np.float32)
    centroids = np.asarray(centroids, np.float32)
    weight = np.asarray(weight, np.float32)
    bias = np.asarray(bias, np.float32)

    if "nc" not in _CACHED:
        _CACHED["nc"] = build_nc()
    nc = _CACHED["nc"]

    in_maps, scale, bias32 = _prep_inputs(x, centroids, weight, bias)
    res = run_bass_kernel_spmd(nc, in_maps, core_ids=list(range(NCORES)))
    return _assemble(
        [res.results[i]["out"] for i in range(NCORES)], scale, bias32
    )


# revision 15
# speedup vs baseline: 1.0878x; 1.0878x over previous
"""AMMLinear (vq_codebook) forward kernel for 8 TRN2 NeuronCores.

Key algebraic fact: the reference's straight-through estimator
    output = real - stop_grad(real - quantized)
is numerically exactly `quantized_output + bias`, so the forward value needs
only:  argmin-distance one-hot  @  fake-quantized lut  + bias.

Distribution: pure data-parallel over the 8192 tokens (1024/core), zero
collectives.  The quantized lut q = clip(round(lut/scale), -127, 127) is
x-independent, computed EXACTLY on host (float64, matching the oracle) and
shipped as two e4m3-exact fp8 planes q = qa16 + qb (qa16 = 16*round(q/16),
qb = q - qa16); the 0/1 one-hots are fp8-exact too.

Gather: token-major fp8 DoubleRow matmuls, psum[tok128, 1024] accumulating
(4 group-pairs) x (a,b) passes with the one-hot pair stationary (reused
across 4 matmuls each).  The PE moving port is ~1KB/partition/213ns, so
this runs at fp16's column rate (exact int8 = 2x fp8 information) with
half the weight loads.  The psum holds exact integer sums (|.|<=8128), so
the drain is a single fp32->fp16 convert-copy on the otherwise-idle Act
engine, and out DMA is fp16 alternating the sync/scalar queues.  The
x-independent  out * scale + bias  epilogue runs on host in fp32.

Scores are ONE fp32 matmul pass per (tile, group) -- exact argmins, a
third of the weight loads of the fp16 3-pass scheme.  Score tiles are
HALF tiles ([128 tok, 512] psum = 1 bank, 4 codebook-groups each) so the
DVE argmax chain frees each bank after only reduce+is_equal of half a
tile.  Early gather units are interleaved into the back half of the score
phase so the PE works through what would otherwise be DVE-pacing stalls.

DMA need-order matters: all queues share ~350GB/s per core, so consts +
bdf stream first, then x tiles, and only then the 8.4MB fp8 lut (first
needed ~25us in), with output DMAs trailing the gather.
"""

import numpy as np

N_TOKENS = 8192
IN_FEAT = 1024
C = 64   # codebooks
KC = 16  # centroids per codebook
S = 16   # subvector length
O = 4096  # out features
NCORES = 8
NLOC = N_TOKENS // NCORES  # 1024 tokens per core
G = 8    # groups of 8 codebooks -> 128-row contraction
GP = 4   # group-pairs (DoubleRow: 2 groups = 256-row contraction)
TT = NLOC // 128  # 8 token tiles
NQ = 4   # o-quarters of 1024 cols (one gather unit each)

_CACHED = {}


def build_nc():
    import concourse.bacc as bacc
    import concourse.mybir as mybir
    import concourse.tile as tile
    from contextlib import ExitStack

    f32 = mybir.dt.float32
    f16 = mybir.dt.float16
    f8 = mybir.dt.float8e4
    AO = mybir.AluOpType
    AF = mybir.ActivationFunctionType
    DR = mybir.MatmulPerfMode.DoubleRow
    X = mybir.AxisListType.X

    nc = bacc.Bacc(
        "TRN2", target_bir_lowering=False, debug=False, num_devices=NCORES
    )

    xf_d = nc.dram_tensor("xf", [128, TT, G, 128], f32, kind="ExternalInput")
    q8a_d = nc.dram_tensor("q8a", [128, NQ, G, 1024], f8, kind="ExternalInput")
    q8b_d = nc.dram_tensor("q8b", [128, NQ, G, 1024], f8, kind="ExternalInput")
    bdf_d = nc.dram_tensor("bdf", [128, G, 128], f32, kind="ExternalInput")
    nc2hl_d = nc.dram_tensor("nc2hl", [2, 1024], f16, kind="ExternalInput")
    or2_d = nc.dram_tensor("or2", [2, 128], f16, kind="ExternalInput")
    kiota_d = nc.dram_tensor("kiota", [128, 1], f16, kind="ExternalInput")
    ioneg_d = nc.dram_tensor("ioneg", [128, 1024], f16, kind="ExternalInput")
    idb_d = nc.dram_tensor("idb", [128, 128], f16, kind="ExternalInput")
    out_d = nc.dram_tensor("out", [NLOC, O], f16, kind="ExternalOutput")

    with ExitStack() as ctx:
        tc = ctx.enter_context(tile.TileContext(nc))
        sb = ctx.enter_context(tc.tile_pool(name="sb", bufs=1))
        # all 8 x tiles resident (4.2MB): a smaller ring spanning the two x
        # DMA queues deadlocks the tile scheduler against the psA slot ring
        sbx = ctx.enter_context(tc.tile_pool(name="sbx", bufs=8))
        sbm = ctx.enter_context(tc.tile_pool(name="sbm", bufs=3))
        sbo = ctx.enter_context(tc.tile_pool(name="sbo", bufs=8))
        sbi = ctx.enter_context(tc.tile_pool(name="sbi", bufs=4))
        psA = ctx.enter_context(tc.tile_pool(name="psA", bufs=3, space="PSUM"))
        psB = ctx.enter_context(tc.tile_pool(name="psB", bufs=2, space="PSUM"))
        psT = ctx.enter_context(tc.tile_pool(name="psT", bufs=1, space="PSUM"))

        # ---------- persistent SBUF ----------
        bdf_sb = sb.tile([128, G, 128], f32)
        nc2hl_sb = sb.tile([2, 1024], f16)
        or2_sb = sb.tile([2, 128], f16)
        kiota_sb = sb.tile([128, 1], f16)
        ioneg_sb = sb.tile([128, 1024], f16)
        idb_sb = sb.tile([128, 128], f16)
        q8a_sb = sb.tile([128, NQ, G, 1024], f8)
        q8b_sb = sb.tile([128, NQ, G, 1024], f8)
        oh8_sb = sb.tile([128, G, NLOC], f8)
        idxT_sb = sb.tile([64, NLOC], f16)

        # ---------- const + input DMAs (global need-order) ----------
        nc.gpsimd.dma_start(or2_sb[:], or2_d[:])
        nc.gpsimd.dma_start(nc2hl_sb[:], nc2hl_d[:])
        nc.gpsimd.dma_start(bdf_sb[:], bdf_d[:])
        nc.gpsimd.dma_start(kiota_sb[:], kiota_d[:])
        nc.gpsimd.dma_start(idb_sb[:], idb_d[:])
        nc.gpsimd.dma_start(ioneg_sb[:], ioneg_d[:])

        xf_tiles = [
            sbx.tile([128, G, 128], f32, tag="xf", name=f"xf{t}")
            for t in range(TT)
        ]
        for t in range(0, TT, 2):
            nc.scalar.dma_start(xf_tiles[t][:], xf_d[:, t])
        for t in range(1, TT, 2):
            nc.sync.dma_start(xf_tiles[t][:], xf_d[:, t])
        for qtr in range(NQ):
            nc.sync.dma_start(q8a_sb[:, qtr], q8a_d[:, qtr])
            nc.sync.dma_start(q8b_sb[:, qtr], q8b_d[:, qtr])

        # ---------- phase S: scores -> first-max index encoding ----------
        idxt_tiles = {}

        def emit_score_half(t, h):
            """Half a token tile: codebook-groups 4h..4h+3 -> 1 psum bank."""
            cc = slice(h * 512, (h + 1) * 512)  # ck columns
            sc_ps = psA.tile([128, 512], f32, tag="sc", name=f"sc{t}_{h}")
            nc.tensor.matmul(
                sc_ps[:], or2_sb[:], nc2hl_sb[:, cc],
                start=True, stop=False, skip_group_check=True,
            )
            for gg in range(4):
                g = 4 * h + gg
                nc.tensor.matmul(
                    sc_ps[:, gg * 128 : (gg + 1) * 128],
                    xf_tiles[t][:, g, :], bdf_sb[:, g, :],
                    start=False, stop=(gg == 3), skip_group_check=True,
                )
            # argmax chain on DVE; psum bank freed right after is_equal
            maxb = sbm.tile([128, 32], f32, tag="maxb", name=f"maxb{t}_{h}")
            nc.vector.tensor_reduce(
                maxb[:], sc_ps[:].rearrange("p (c k) -> p c k", k=KC),
                axis=X, op=AO.max,
            )
            mask = sbm.tile([128, 512], f16, tag="mask", name=f"mask{t}_{h}")
            nc.vector.tensor_tensor(
                mask[:].rearrange("p (c k) -> p c k", k=KC),
                sc_ps[:].rearrange("p (c k) -> p c k", k=KC),
                maxb[:].rearrange("p (c u) -> p c u", u=1).broadcast_to((128, 32, KC)),
                op=AO.is_equal,
            )
            # iv = mask*64 + (15-k): max picks the first (smallest-k) hit
            nc.vector.scalar_tensor_tensor(
                mask[:], mask[:], 64.0, ioneg_sb[:, cc], op0=AO.mult, op1=AO.add
            )
            if h == 0:
                idxt_tiles[t] = sbi.tile([128, C], f16, tag="idxt", name=f"idxt{t}")
            nc.vector.tensor_reduce(
                idxt_tiles[t][:, h * 32 : (h + 1) * 32],
                mask[:].rearrange("p (c k) -> p c k", k=KC),
                axis=X, op=AO.max,
            )

        def emit_score(t):
            emit_score_half(t, 0)
            emit_score_half(t, 1)

        # deferred: transpose tile t's index row into idxT (PE + DVE copy);
        # emitted a few slots after the chain so the PE never waits on it
        def emit_tp(t):
            tok = slice(t * 128, (t + 1) * 128)
            tp_ps = psT.tile([64, 128], f16, tag="tp", name=f"tp{t}")
            nc.tensor.transpose(tp_ps[:], idxt_tiles[t][:], idb_sb[:])
            nc.vector.tensor_copy(idxT_sb[:, tok], tp_ps[:])

        # one-hot expansion for (group g, token half h), straight to fp8
        def emit_oh(g, h):
            cols = slice(h * 512, (h + 1) * 512)
            idxb = sbi.tile([128, 512], f16, tag="idxb", name=f"idxb{g}_{h}")
            nc.gpsimd.dma_start(
                idxb[:],
                idxT_sb[g * 8 : (g + 1) * 8, cols]
                .rearrange("j (n u) -> j u n", u=1)
                .broadcast_to((8, KC, 512)),
            )
            nc.vector.tensor_tensor(
                oh8_sb[:, g, cols], idxb[:],
                kiota_sb[:, 0:1].broadcast_to((128, 512)),
                op=AO.is_equal,
            )

        # ---------- phase G: token-major DoubleRow gather units ----------
        def emit_gunit(t, qtr, u):
            tok = slice(t * 128, (t + 1) * 128)
            ps = psB.tile([128, 1024], f32, tag="gu", name=f"gu{t}_{qtr}")
            for gp in range(GP):
                lhs = oh8_sb[:, 2 * gp : 2 * gp + 2, tok]
                for ch in range(2):
                    oc = slice(ch * 512, (ch + 1) * 512)
                    nc.tensor.matmul(
                        ps[:, oc], lhs, q8a_sb[:, qtr, 2 * gp : 2 * gp + 2, oc],
                        start=(gp == 0), stop=False,
                        perf_mode=DR, skip_group_check=True,
                    )
                    nc.tensor.matmul(
                        ps[:, oc], lhs, q8b_sb[:, qtr, 2 * gp : 2 * gp + 2, oc],
                        start=False, stop=(gp == GP - 1),
                        perf_mode=DR, skip_group_check=True,
                    )
            o16 = sbo.tile([128, 1024], f16, tag="o16", name=f"o16_{t}_{qtr}")
            # drains on the otherwise-idle Act engine (GpSimd can't read
            # PSUM); the second-to-last goes to DVE so the tail overlaps
            if u == 30:
                nc.vector.tensor_copy(o16[:], ps[:])
            else:
                nc.scalar.activation(o16[:], ps[:], AF.Identity, bias=0.0, scale=1.0)
            eng = nc.sync if u % 2 == 0 else nc.scalar
            eng.dma_start(out_d[tok, qtr * 1024 : (qtr + 1) * 1024], o16[:])

        # ---------- interleaved emission (PE and DVE queues in-order) ----
        # gather units for tiles 0/1 interleave into the back of the score
        # phase: they fill what would otherwise be PE stalls waiting on the
        # DVE chain to recycle score-psum banks
        emit_score(0); emit_score(1); emit_score(2); emit_score(3)
        emit_tp(0); emit_tp(1); emit_tp(2); emit_tp(3)
        for g in range(G):
            emit_oh(g, 0)
        emit_gunit(0, 0, 0)
        emit_score(4)
        emit_gunit(0, 1, 1)
        emit_score(5)
        emit_gunit(0, 2, 2)
        emit_score(6)
        emit_gunit(0, 3, 3)
        emit_score(7)
        emit_gunit(1, 0, 4)
        emit_tp(4)
        emit_gunit(1, 1, 5)
        emit_tp(5)
        emit_gunit(1, 2, 6)
        emit_tp(6)
        emit_gunit(1, 3, 7)
        emit_tp(7)
        for g in range(G):
            emit_oh(g, 1)
        u = 8
        for t in range(2, TT):
            for qtr in range(NQ):
                emit_gunit(t, qtr, u)
                u += 1

    nc.compile()
    return nc


def _consts():
    kiota = (79.0 - np.arange(128, dtype=np.float32) % KC).reshape(128, 1).astype(np.float16)
    ioneg = np.tile(
        15.0 - (np.arange(1024, dtype=np.float32) % KC), (128, 1)
    ).astype(np.float16)
    idb = np.eye(128, dtype=np.float16)
    return kiota, ioneg, idb


def _prep_inputs(x, centroids, weight, bias):
    """Host-side shard/layout prep, exact int8 fake-quant lut, fp8 split."""
    import ml_dtypes

    kiota, ioneg, idb = _consts()
    # block-diagonal centroids^T: bd[16j+s, g, 16j+k] = centroids[8g+j, k, s]
    bd = np.zeros((128, G, 128), np.float32)
    for g in range(G):
        for j in range(8):
            bd[16 * j : 16 * (j + 1), g, 16 * j : 16 * (j + 1)] = centroids[
                8 * g + j
            ].T
    nc2 = (-0.5 * (centroids.astype(np.float64) ** 2).sum(-1)).astype(
        np.float32
    ).reshape(1, C * KC)
    nc2h = nc2.astype(np.float16)
    nc2l = (nc2 - nc2h.astype(np.float32)).astype(np.float16)
    nc2hl = np.concatenate([nc2h, nc2l], axis=0)
    or2 = np.ones((2, 128), np.float16)

    # exact int8 fake-quant lut (float64, matching the oracle) + fp8 split
    lut = np.einsum(
        "cks,cso->cko", centroids.astype(np.float64), weight.astype(np.float64)
    )
    amax = np.abs(lut).max()
    scale = np.float32(amax / 127.0)
    q = np.clip(np.round(lut / (amax / 127.0)), -127.0, 127.0)
    qa16 = 16.0 * np.round(q / 16.0)   # multiples of 16, |.| <= 128
    qb = q - qa16                      # ints, |.| <= 8

    def pack(v):
        v = v.reshape(G, 8, KC, O)       # [g, j, k, o]
        v = v.transpose(1, 2, 0, 3)      # [j, k, g, o] -> p = 16j+k
        v = v.reshape(128, G, NQ, 1024)  # [p, g, qtr, oc]
        v = v.transpose(0, 2, 1, 3)      # [p, qtr, g, oc]
        return np.ascontiguousarray(v).astype(ml_dtypes.float8_e4m3)

    common = dict(
        q8a=pack(qa16), q8b=pack(qb), bdf=bd, nc2hl=nc2hl,
        or2=or2, kiota=kiota, ioneg=ioneg, idb=idb,
    )
    in_maps = []
    for i in range(NCORES):
        xs = x[i * NLOC : (i + 1) * NLOC, :]  # (1024, 1024)
        xf = np.ascontiguousarray(
            xs.T.reshape(G, 128, TT, 128).transpose(1, 2, 0, 3)
        )  # [p, t, g, n] float32
        m = dict(common)
        m.update(xf=xf)
        in_maps.append(m)
    return in_maps, scale, np.asarray(bias, np.float32)


def _assemble(per_core_outs, scale, bias32):
    out = np.concatenate(
        [np.asarray(o).astype(np.float32) for o in per_core_outs], axis=0
    )
    return out * scale + bias32[None, :]


def kernel(x, centroids, weight, inverse_temperature_logit, bias, **_):
    from concourse.bass_utils import run_bass_kernel_spmd

    x = np.asarray(x, np.float32)
    centroids = np.asarray(centroids, np.float32)
    weight = np.asarray(weight, np.float32)
    bias = np.asarray(bias, np.float32)

    if "nc" not in _CACHED:
        _CACHED["nc"] = build_nc()
    nc = _CACHED["nc"]

    in_maps, scale, bias32 = _prep_inputs(x, centroids, weight, bias)
    res = run_bass_kernel_spmd(nc, in_maps, core_ids=list(range(NCORES)))
    return _assemble(
        [res.results[i]["out"] for i in range(NCORES)], scale, bias32
    )


# revision 16
# speedup vs baseline: 1.0956x; 1.0071x over previous
"""AMMLinear (vq_codebook) forward kernel for 8 TRN2 NeuronCores.

Key algebraic fact: the reference's straight-through estimator
    output = real - stop_grad(real - quantized)
is numerically exactly `quantized_output + bias`, so the forward value needs
only:  argmin-distance one-hot  @  fake-quantized lut  + bias.

Distribution: pure data-parallel over the 8192 tokens (1024/core), zero
collectives.  The quantized lut q = clip(round(lut/scale), -127, 127) is
x-independent, computed EXACTLY on host (float64, matching the oracle) and
shipped as two e4m3-exact fp8 planes q = qa16 + qb (qa16 = 16*round(q/16),
qb = q - qa16); the 0/1 one-hots are fp8-exact too.

Gather: token-major fp8 DoubleRow matmuls, psum[tok128, 1024] accumulating
(4 group-pairs) x (a,b) passes with the one-hot pair stationary (reused
across 4 matmuls each).  The PE moving port is ~1KB/partition/213ns, so
this runs at fp16's column rate (exact int8 = 2x fp8 information) with
half the weight loads.  The psum holds exact integer sums (|.|<=8128), so
the drain is a single fp32->fp16 convert-copy on the otherwise-idle Act
engine, and out DMA is fp16 alternating the sync/scalar queues.  The
x-independent  out * scale + bias  epilogue runs on host in fp32.

Scores are ONE fp32 matmul pass per (tile, group) -- exact argmins, a
third of the weight loads of the fp16 3-pass scheme.  Score tiles are
HALF tiles ([128 tok, 512] psum = 1 bank, 4 codebook-groups each) so the
DVE argmax chain frees each bank after only reduce+is_equal of half a
tile.  Early gather units are interleaved into the back half of the score
phase so the PE works through what would otherwise be DVE-pacing stalls.

DMA need-order matters: all queues share ~350GB/s per core, so consts +
bdf stream first, then x tiles, and only then the 8.4MB fp8 lut (first
needed ~25us in), with output DMAs trailing the gather.
"""

import numpy as np

N_TOKENS = 8192
IN_FEAT = 1024
C = 64   # codebooks
KC = 16  # centroids per codebook
S = 16   # subvector length
O = 4096  # out features
NCORES = 8
NLOC = N_TOKENS // NCORES  # 1024 tokens per core
G = 8    # groups of 8 codebooks -> 128-row contraction
GP = 4   # group-pairs (DoubleRow: 2 groups = 256-row contraction)
TT = NLOC // 128  # 8 token tiles
NQ = 4   # o-quarters of 1024 cols (one gather unit each)

_CACHED = {}


def build_nc():
    import concourse.bacc as bacc
    import concourse.mybir as mybir
    import concourse.tile as tile
    from contextlib import ExitStack

    f32 = mybir.dt.float32
    f16 = mybir.dt.float16
    f8 = mybir.dt.float8e4
    AO = mybir.AluOpType
    AF = mybir.ActivationFunctionType
    DR = mybir.MatmulPerfMode.DoubleRow
    X = mybir.AxisListType.X

    nc = bacc.Bacc(
        "TRN2", target_bir_lowering=False, debug=False, num_devices=NCORES
    )

    xf_d = nc.dram_tensor("xf", [128, TT, G, 128], f32, kind="ExternalInput")
    q8a_d = nc.dram_tensor("q8a", [128, NQ, G, 1024], f8, kind="ExternalInput")
    q8b_d = nc.dram_tensor("q8b", [128, NQ, G, 1024], f8, kind="ExternalInput")
    bdf_d = nc.dram_tensor("bdf", [128, G, 128], f32, kind="ExternalInput")
    nc2hl_d = nc.dram_tensor("nc2hl", [2, 1024], f16, kind="ExternalInput")
    or2_d = nc.dram_tensor("or2", [2, 128], f16, kind="ExternalInput")
    kiota_d = nc.dram_tensor("kiota", [128, 1], f16, kind="ExternalInput")
    ioneg_d = nc.dram_tensor("ioneg", [128, 1024], f16, kind="ExternalInput")
    idb_d = nc.dram_tensor("idb", [128, 128], f16, kind="ExternalInput")
    out_d = nc.dram_tensor("out", [NLOC, O], f16, kind="ExternalOutput")

    with ExitStack() as ctx:
        tc = ctx.enter_context(tile.TileContext(nc))
        sb = ctx.enter_context(tc.tile_pool(name="sb", bufs=1))
        # all 8 x tiles resident (4.2MB): a smaller ring spanning the two x
        # DMA queues deadlocks the tile scheduler against the psA slot ring
        sbx = ctx.enter_context(tc.tile_pool(name="sbx", bufs=8))
        sbm = ctx.enter_context(tc.tile_pool(name="sbm", bufs=3))
        sbo = ctx.enter_context(tc.tile_pool(name="sbo", bufs=8))
        sbi = ctx.enter_context(tc.tile_pool(name="sbi", bufs=4))
        psA = ctx.enter_context(tc.tile_pool(name="psA", bufs=3, space="PSUM"))
        psB = ctx.enter_context(tc.tile_pool(name="psB", bufs=2, space="PSUM"))
        psT = ctx.enter_context(tc.tile_pool(name="psT", bufs=1, space="PSUM"))

        # ---------- persistent SBUF ----------
        bdf_sb = sb.tile([128, G, 128], f32)
        nc2hl_sb = sb.tile([2, 1024], f16)
        or2_sb = sb.tile([2, 128], f16)
        kiota_sb = sb.tile([128, 1], f16)
        ioneg_sb = sb.tile([128, 1024], f16)
        idb_sb = sb.tile([128, 128], f16)
        q8a_sb = sb.tile([128, NQ, G, 1024], f8)
        q8b_sb = sb.tile([128, NQ, G, 1024], f8)
        oh8_sb = sb.tile([128, G, NLOC], f8)
        idxT_sb = sb.tile([64, NLOC], f16)

        # ---------- const + input DMAs (global need-order) ----------
        nc.gpsimd.dma_start(or2_sb[:], or2_d[:])
        nc.gpsimd.dma_start(nc2hl_sb[:], nc2hl_d[:])
        nc.gpsimd.dma_start(bdf_sb[:], bdf_d[:])
        nc.gpsimd.dma_start(kiota_sb[:], kiota_d[:])
        nc.gpsimd.dma_start(idb_sb[:], idb_d[:])
        nc.gpsimd.dma_start(ioneg_sb[:], ioneg_d[:])

        xf_tiles = [
            sbx.tile([128, G, 128], f32, tag="xf", name=f"xf{t}")
            for t in range(TT)
        ]
        for t in range(0, TT, 2):
            nc.scalar.dma_start(xf_tiles[t][:], xf_d[:, t])
        for t in range(1, TT, 2):
            nc.sync.dma_start(xf_tiles[t][:], xf_d[:, t])
        for qtr in range(NQ):
            nc.sync.dma_start(q8a_sb[:, qtr], q8a_d[:, qtr])
            nc.sync.dma_start(q8b_sb[:, qtr], q8b_d[:, qtr])

        # ---------- phase S: scores -> first-max index encoding ----------
        idxt_tiles = {}

        def emit_score_half(t, h):
            """Half a token tile: codebook-groups 4h..4h+3 -> 1 psum bank."""
            cc = slice(h * 512, (h + 1) * 512)  # ck columns
            sc_ps = psA.tile([128, 512], f32, tag="sc", name=f"sc{t}_{h}")
            nc.tensor.matmul(
                sc_ps[:], or2_sb[:], nc2hl_sb[:, cc],
                start=True, stop=False, skip_group_check=True,
            )
            for gg in range(4):
                g = 4 * h + gg
                nc.tensor.matmul(
                    sc_ps[:, gg * 128 : (gg + 1) * 128],
                    xf_tiles[t][:, g, :], bdf_sb[:, g, :],
                    start=False, stop=(gg == 3), skip_group_check=True,
                )
            # argmax chain on DVE; psum bank freed right after is_equal
            maxb = sbm.tile([128, 32], f32, tag="maxb", name=f"maxb{t}_{h}")
            nc.vector.tensor_reduce(
                maxb[:], sc_ps[:].rearrange("p (c k) -> p c k", k=KC),
                axis=X, op=AO.max,
            )
            mask = sbm.tile([128, 512], f16, tag="mask", name=f"mask{t}_{h}")
            nc.vector.tensor_tensor(
                mask[:].rearrange("p (c k) -> p c k", k=KC),
                sc_ps[:].rearrange("p (c k) -> p c k", k=KC),
                maxb[:].rearrange("p (c u) -> p c u", u=1).broadcast_to((128, 32, KC)),
                op=AO.is_equal,
            )
            # iv = mask*64 + (15-k): max picks the first (smallest-k) hit
            nc.vector.scalar_tensor_tensor(
                mask[:], mask[:], 64.0, ioneg_sb[:, cc], op0=AO.mult, op1=AO.add
            )
            if h == 0:
                idxt_tiles[t] = sbi.tile([128, C], f16, tag="idxt", name=f"idxt{t}")
            nc.vector.tensor_reduce(
                idxt_tiles[t][:, h * 32 : (h + 1) * 32],
                mask[:].rearrange("p (c k) -> p c k", k=KC),
                axis=X, op=AO.max,
            )

        def emit_score(t):
            emit_score_half(t, 0)
            emit_score_half(t, 1)

        # deferred: transpose tile t's index row into idxT (PE + DVE copy);
        # emitted a few slots after the chain so the PE never waits on it
        def emit_tp(t):
            tok = slice(t * 128, (t + 1) * 128)
            tp_ps = psT.tile([64, 128], f16, tag="tp", name=f"tp{t}")
            nc.tensor.transpose(tp_ps[:], idxt_tiles[t][:], idb_sb[:])
            nc.vector.tensor_copy(idxT_sb[:, tok], tp_ps[:])

        # one-hot expansion for (group g, token half h), straight to fp8
        def emit_oh(g, h):
            cols = slice(h * 512, (h + 1) * 512)
            idxb = sbi.tile([128, 512], f16, tag="idxb", name=f"idxb{g}_{h}")
            nc.gpsimd.dma_start(
                idxb[:],
                idxT_sb[g * 8 : (g + 1) * 8, cols]
                .rearrange("j (n u) -> j u n", u=1)
                .broadcast_to((8, KC, 512)),
            )
            nc.vector.tensor_tensor(
                oh8_sb[:, g, cols], idxb[:],
                kiota_sb[:, 0:1].broadcast_to((128, 512)),
                op=AO.is_equal,
            )

        # ---------- phase G: token-major DoubleRow gather units ----------
        def emit_gunit(t, qtr, u):
            tok = slice(t * 128, (t + 1) * 128)
            ps = psB.tile([128, 1024], f32, tag="gu", name=f"gu{t}_{qtr}")
            for gp in range(GP):
                lhs = oh8_sb[:, 2 * gp : 2 * gp + 2, tok]
                for ch in range(2):
                    oc = slice(ch * 512, (ch + 1) * 512)
                    nc.tensor.matmul(
                        ps[:, oc], lhs, q8a_sb[:, qtr, 2 * gp : 2 * gp + 2, oc],
                        start=(gp == 0), stop=False,
                        perf_mode=DR, skip_group_check=True,
                    )
                    nc.tensor.matmul(
                        ps[:, oc], lhs, q8b_sb[:, qtr, 2 * gp : 2 * gp + 2, oc],
                        start=False, stop=(gp == GP - 1),
                        perf_mode=DR, skip_group_check=True,
                    )
            o16 = sbo.tile([128, 1024], f16, tag="o16", name=f"o16_{t}_{qtr}")
            # drains on the otherwise-idle Act engine (GpSimd can't read
            # PSUM); the second-to-last goes to DVE so the tail overlaps
            if u == 30:
                nc.vector.tensor_copy(o16[:], ps[:])
            else:
                nc.scalar.activation(o16[:], ps[:], AF.Identity, bias=0.0, scale=1.0)
            eng = nc.sync if u % 2 == 0 else nc.scalar
            eng.dma_start(out_d[tok, qtr * 1024 : (qtr + 1) * 1024], o16[:])

        # ---------- interleaved emission (PE and DVE queues in-order) ----
        # gather units for tiles 0/1 interleave into the back of the score
        # phase: they fill what would otherwise be PE stalls waiting on the
        # DVE chain to recycle score-psum banks
        emit_score(0); emit_score(1); emit_score(2); emit_score(3)
        emit_tp(0); emit_tp(1); emit_tp(2); emit_tp(3)
        for g in range(G):
            emit_oh(g, 0)
        # tiles 0/1 alternate so early units consume q8 quarters at half
        # rate (the fp8 lut is still streaming in behind the x tiles)
        emit_gunit(0, 0, 0)
        emit_score(4)
        emit_gunit(1, 0, 1)
        emit_score(5)
        emit_gunit(0, 1, 2)
        emit_score(6)
        emit_gunit(1, 1, 3)
        emit_score(7)
        emit_gunit(0, 2, 4)
        emit_tp(4)
        emit_gunit(1, 2, 5)
        emit_tp(5)
        emit_gunit(0, 3, 6)
        emit_tp(6)
        emit_gunit(1, 3, 7)
        emit_tp(7)
        for g in range(G):
            emit_oh(g, 1)
        u = 8
        for t in range(2, TT):
            for qtr in range(NQ):
                emit_gunit(t, qtr, u)
                u += 1

    nc.compile()
    return nc


def _consts():
    kiota = (79.0 - np.arange(128, dtype=np.float32) % KC).reshape(128, 1).astype(np.float16)
    ioneg = np.tile(
        15.0 - (np.arange(1024, dtype=np.float32) % KC), (128, 1)
    ).astype(np.float16)
    idb = np.eye(128, dtype=np.float16)
    return kiota, ioneg, idb


def _prep_inputs(x, centroids, weight, bias):
    """Host-side shard/layout prep, exact int8 fake-quant lut, fp8 split."""
    import ml_dtypes

    kiota, ioneg, idb = _consts()
    # block-diagonal centroids^T: bd[16j+s, g, 16j+k] = centroids[8g+j, k, s]
    bd = np.zeros((128, G, 128), np.float32)
    for g in range(G):
        for j in range(8):
            bd[16 * j : 16 * (j + 1), g, 16 * j : 16 * (j + 1)] = centroids[
                8 * g + j
            ].T
    nc2 = (-0.5 * (centroids.astype(np.float64) ** 2).sum(-1)).astype(
        np.float32
    ).reshape(1, C * KC)
    nc2h = nc2.astype(np.float16)
    nc2l = (nc2 - nc2h.astype(np.float32)).astype(np.float16)
    nc2hl = np.concatenate([nc2h, nc2l], axis=0)
    or2 = np.ones((2, 128), np.float16)

    # exact int8 fake-quant lut (float64, matching the oracle) + fp8 split
    lut = np.einsum(
        "cks,cso->cko", centroids.astype(np.float64), weight.astype(np.float64)
    )
    amax = np.abs(lut).max()
    scale = np.float32(amax / 127.0)
    q = np.clip(np.round(lut / (amax / 127.0)), -127.0, 127.0)
    qa16 = 16.0 * np.round(q / 16.0)   # multiples of 16, |.| <= 128
    qb = q - qa16                      # ints, |.| <= 8

    def pack(v):
        v = v.reshape(G, 8, KC, O)       # [g, j, k, o]
        v = v.transpose(1, 2, 0, 3)      # [j, k, g, o] -> p = 16j+k
        v = v.reshape(128, G, NQ, 1024)  # [p, g, qtr, oc]
        v = v.transpose(0, 2, 1, 3)      # [p, qtr, g, oc]
        return np.ascontiguousarray(v).astype(ml_dtypes.float8_e4m3)

    common = dict(
        q8a=pack(qa16), q8b=pack(qb), bdf=bd, nc2hl=nc2hl,
        or2=or2, kiota=kiota, ioneg=ioneg, idb=idb,
    )
    in_maps = []
    for i in range(NCORES):
        xs = x[i * NLOC : (i + 1) * NLOC, :]  # (1024, 1024)
        xf = np.ascontiguousarray(
            xs.T.reshape(G, 128, TT, 128).transpose(1, 2, 0, 3)
        )  # [p, t, g, n] float32
        m = dict(common)
        m.update(xf=xf)
        in_maps.append(m)
    return in_maps, scale, np.asarray(bias, np.float32)


def _assemble(per_core_outs, scale, bias32):
    out = np.concatenate(
        [np.asarray(o).astype(np.float32) for o in per_core_outs], axis=0
    )
    return out * scale + bias32[None, :]


def kernel(x, centroids, weight, inverse_temperature_logit, bias, **_):
    from concourse.bass_utils import run_bass_kernel_spmd

    x = np.asarray(x, np.float32)
    centroids = np.asarray(centroids, np.float32)
    weight = np.asarray(weight, np.float32)
    bias = np.asarray(bias, np.float32)

    if "nc" not in _CACHED:
        _CACHED["nc"] = build_nc()
    nc = _CACHED["nc"]

    in_maps, scale, bias32 = _prep_inputs(x, centroids, weight, bias)
    res = run_bass_kernel_spmd(nc, in_maps, core_ids=list(range(NCORES)))
    return _assemble(
        [res.results[i]["out"] for i in range(NCORES)], scale, bias32
    )


# revision 17
# speedup vs baseline: 1.0978x; 1.0020x over previous
"""AMMLinear (vq_codebook) forward kernel for 8 TRN2 NeuronCores.

Key algebraic fact: the reference's straight-through estimator
    output = real - stop_grad(real - quantized)
is numerically exactly `quantized_output + bias`, so the forward value needs
only:  argmin-distance one-hot  @  fake-quantized lut  + bias.

Distribution: pure data-parallel over the 8192 tokens (1024/core), zero
collectives.  The quantized lut q = clip(round(lut/scale), -127, 127) is
x-independent, computed EXACTLY on host (float64, matching the oracle) and
shipped as two e4m3-exact fp8 planes q = qa16 + qb (qa16 = 16*round(q/16),
qb = q - qa16); the 0/1 one-hots are fp8-exact too.

Gather: token-major fp8 DoubleRow matmuls, psum[tok128, 1024] accumulating
(4 group-pairs) x (a,b) passes with the one-hot pair stationary (reused
across 4 matmuls each).  The PE moving port is ~1KB/partition/213ns, so
this runs at fp16's column rate (exact int8 = 2x fp8 information) with
half the weight loads.  The psum holds exact integer sums (|.|<=8128), so
the drain is a single fp32->fp16 convert-copy on the otherwise-idle Act
engine, and out DMA is fp16 alternating the sync/scalar queues.  The
x-independent  out * scale + bias  epilogue runs on host in fp32.

Scores are ONE fp32 matmul pass per (tile, group) -- exact argmins, a
third of the weight loads of the fp16 3-pass scheme.  Score tiles are
HALF tiles ([128 tok, 512] psum = 1 bank, 4 codebook-groups each) so the
DVE argmax chain frees each bank after only reduce+is_equal of half a
tile.  Early gather units are interleaved into the back half of the score
phase so the PE works through what would otherwise be DVE-pacing stalls.

DMA need-order matters: all queues share ~350GB/s per core, so consts +
bdf stream first, then x tiles, and only then the 8.4MB fp8 lut (first
needed ~25us in), with output DMAs trailing the gather.
"""

import numpy as np

N_TOKENS = 8192
IN_FEAT = 1024
C = 64   # codebooks
KC = 16  # centroids per codebook
S = 16   # subvector length
O = 4096  # out features
NCORES = 8
NLOC = N_TOKENS // NCORES  # 1024 tokens per core
G = 8    # groups of 8 codebooks -> 128-row contraction
GP = 4   # group-pairs (DoubleRow: 2 groups = 256-row contraction)
TT = NLOC // 128  # 8 token tiles
NQ = 4   # o-quarters of 1024 cols (one gather unit each)

_CACHED = {}


def build_nc():
    import concourse.bacc as bacc
    import concourse.mybir as mybir
    import concourse.tile as tile
    from contextlib import ExitStack

    f32 = mybir.dt.float32
    f16 = mybir.dt.float16
    f8 = mybir.dt.float8e4
    AO = mybir.AluOpType
    AF = mybir.ActivationFunctionType
    DR = mybir.MatmulPerfMode.DoubleRow
    X = mybir.AxisListType.X

    nc = bacc.Bacc(
        "TRN2", target_bir_lowering=False, debug=False, num_devices=NCORES
    )

    xf_d = nc.dram_tensor("xf", [128, TT, G, 128], f32, kind="ExternalInput")
    q8a_d = nc.dram_tensor("q8a", [128, NQ, G, 1024], f8, kind="ExternalInput")
    q8b_d = nc.dram_tensor("q8b", [128, NQ, G, 1024], f8, kind="ExternalInput")
    bdf_d = nc.dram_tensor("bdf", [128, G, 128], f32, kind="ExternalInput")
    nc2hl_d = nc.dram_tensor("nc2hl", [2, 1024], f16, kind="ExternalInput")
    or2_d = nc.dram_tensor("or2", [2, 128], f16, kind="ExternalInput")
    kiota_d = nc.dram_tensor("kiota", [128, 1], f16, kind="ExternalInput")
    ioneg_d = nc.dram_tensor("ioneg", [128, 1024], f16, kind="ExternalInput")
    idb_d = nc.dram_tensor("idb", [128, 128], f16, kind="ExternalInput")
    out_d = nc.dram_tensor("out", [NLOC, O], f16, kind="ExternalOutput")

    with ExitStack() as ctx:
        tc = ctx.enter_context(tile.TileContext(nc))
        sb = ctx.enter_context(tc.tile_pool(name="sb", bufs=1))
        # all 8 x tiles resident (4.2MB): a smaller ring spanning the two x
        # DMA queues deadlocks the tile scheduler against the psA slot ring
        sbx = ctx.enter_context(tc.tile_pool(name="sbx", bufs=8))
        sbm = ctx.enter_context(tc.tile_pool(name="sbm", bufs=3))
        sbo = ctx.enter_context(tc.tile_pool(name="sbo", bufs=8))
        sbi = ctx.enter_context(tc.tile_pool(name="sbi", bufs=4))
        psA = ctx.enter_context(tc.tile_pool(name="psA", bufs=3, space="PSUM"))
        psB = ctx.enter_context(tc.tile_pool(name="psB", bufs=2, space="PSUM"))
        psT = ctx.enter_context(tc.tile_pool(name="psT", bufs=1, space="PSUM"))

        # ---------- persistent SBUF ----------
        bdf_sb = sb.tile([128, G, 128], f32)
        nc2hl_sb = sb.tile([2, 1024], f16)
        or2_sb = sb.tile([2, 128], f16)
        kiota_sb = sb.tile([128, 1], f16)
        ioneg_sb = sb.tile([128, 1024], f16)
        idb_sb = sb.tile([128, 128], f16)
        q8a_sb = sb.tile([128, NQ, G, 1024], f8)
        q8b_sb = sb.tile([128, NQ, G, 1024], f8)
        oh8_sb = sb.tile([128, G, NLOC], f8)
        idxT_sb = sb.tile([64, NLOC], f16)

        # ---------- const + input DMAs (global need-order) ----------
        nc.gpsimd.dma_start(or2_sb[:], or2_d[:])
        nc.gpsimd.dma_start(nc2hl_sb[:], nc2hl_d[:])
        nc.gpsimd.dma_start(bdf_sb[:], bdf_d[:])
        nc.gpsimd.dma_start(kiota_sb[:], kiota_d[:])
        nc.gpsimd.dma_start(idb_sb[:], idb_d[:])
        nc.gpsimd.dma_start(ioneg_sb[:], ioneg_d[:])

        xf_tiles = [
            sbx.tile([128, G, 128], f32, tag="xf", name=f"xf{t}")
            for t in range(TT)
        ]
        for t in range(0, TT, 2):
            nc.scalar.dma_start(xf_tiles[t][:], xf_d[:, t])
        for t in range(1, TT, 2):
            nc.sync.dma_start(xf_tiles[t][:], xf_d[:, t])
        for qtr in range(NQ):
            nc.sync.dma_start(q8a_sb[:, qtr], q8a_d[:, qtr])
            nc.sync.dma_start(q8b_sb[:, qtr], q8b_d[:, qtr])

        # ---------- phase S: scores -> first-max index encoding ----------
        idxt_tiles = {}

        def emit_score_half(t, h):
            """Half a token tile: codebook-groups 4h..4h+3 -> 1 psum bank."""
            cc = slice(h * 512, (h + 1) * 512)  # ck columns
            sc_ps = psA.tile([128, 512], f32, tag="sc", name=f"sc{t}_{h}")
            nc.tensor.matmul(
                sc_ps[:], or2_sb[:], nc2hl_sb[:, cc],
                start=True, stop=False, skip_group_check=True,
            )
            for gg in range(4):
                g = 4 * h + gg
                nc.tensor.matmul(
                    sc_ps[:, gg * 128 : (gg + 1) * 128],
                    xf_tiles[t][:, g, :], bdf_sb[:, g, :],
                    start=False, stop=(gg == 3), skip_group_check=True,
                )
            # argmax chain on DVE; psum bank freed right after is_equal
            maxb = sbm.tile([128, 32], f32, tag="maxb", name=f"maxb{t}_{h}")
            nc.vector.tensor_reduce(
                maxb[:], sc_ps[:].rearrange("p (c k) -> p c k", k=KC),
                axis=X, op=AO.max,
            )
            mask = sbm.tile([128, 512], f16, tag="mask", name=f"mask{t}_{h}")
            nc.vector.tensor_tensor(
                mask[:].rearrange("p (c k) -> p c k", k=KC),
                sc_ps[:].rearrange("p (c k) -> p c k", k=KC),
                maxb[:].rearrange("p (c u) -> p c u", u=1).broadcast_to((128, 32, KC)),
                op=AO.is_equal,
            )
            # iv = mask*64 + (15-k): max picks the first (smallest-k) hit
            nc.vector.scalar_tensor_tensor(
                mask[:], mask[:], 64.0, ioneg_sb[:, cc], op0=AO.mult, op1=AO.add
            )
            if h == 0:
                idxt_tiles[t] = sbi.tile([128, C], f16, tag="idxt", name=f"idxt{t}")
            nc.vector.tensor_reduce(
                idxt_tiles[t][:, h * 32 : (h + 1) * 32],
                mask[:].rearrange("p (c k) -> p c k", k=KC),
                axis=X, op=AO.max,
            )

        def emit_score(t):
            emit_score_half(t, 0)
            emit_score_half(t, 1)

        # deferred: transpose tile t's index row into idxT (PE + DVE copy);
        # emitted a few slots after the chain so the PE never waits on it
        def emit_tp(t):
            tok = slice(t * 128, (t + 1) * 128)
            tp_ps = psT.tile([64, 128], f16, tag="tp", name=f"tp{t}")
            nc.tensor.transpose(tp_ps[:], idxt_tiles[t][:], idb_sb[:])
            nc.vector.tensor_copy(idxT_sb[:, tok], tp_ps[:])

        # one-hot expansion for ONE token tile: 8 narrow broadcasts fill a
        # [128, G, 128] index tile, one batched is_equal writes all groups --
        # so gathers for tile t need only tile t's argmax chain
        def emit_oh(t):
            cols = slice(t * 128, (t + 1) * 128)
            idxb = sbi.tile([128, G, 128], f16, tag="idxb", name=f"idxb{t}")
            for g in range(G):
                nc.gpsimd.dma_start(
                    idxb[:, g, :],
                    idxT_sb[g * 8 : (g + 1) * 8, cols]
                    .rearrange("j (n u) -> j u n", u=1)
                    .broadcast_to((8, KC, 128)),
                )
            nc.vector.tensor_tensor(
                oh8_sb[:, :, cols], idxb[:],
                kiota_sb[:, 0:1].broadcast_to((128, G, 128)),
                op=AO.is_equal,
            )

        # ---------- phase G: token-major DoubleRow gather units ----------
        def emit_gunit(t, qtr, u):
            tok = slice(t * 128, (t + 1) * 128)
            ps = psB.tile([128, 1024], f32, tag="gu", name=f"gu{t}_{qtr}")
            for gp in range(GP):
                lhs = oh8_sb[:, 2 * gp : 2 * gp + 2, tok]
                for ch in range(2):
                    oc = slice(ch * 512, (ch + 1) * 512)
                    nc.tensor.matmul(
                        ps[:, oc], lhs, q8a_sb[:, qtr, 2 * gp : 2 * gp + 2, oc],
                        start=(gp == 0), stop=False,
                        perf_mode=DR, skip_group_check=True,
                    )
                    nc.tensor.matmul(
                        ps[:, oc], lhs, q8b_sb[:, qtr, 2 * gp : 2 * gp + 2, oc],
                        start=False, stop=(gp == GP - 1),
                        perf_mode=DR, skip_group_check=True,
                    )
            o16 = sbo.tile([128, 1024], f16, tag="o16", name=f"o16_{t}_{qtr}")
            # drains on the otherwise-idle Act engine (GpSimd can't read
            # PSUM); the second-to-last goes to DVE so the tail overlaps
            if u == 30:
                nc.vector.tensor_copy(o16[:], ps[:])
            else:
                nc.scalar.activation(o16[:], ps[:], AF.Identity, bias=0.0, scale=1.0)
            eng = nc.sync if u % 2 == 0 else nc.scalar
            eng.dma_start(out_d[tok, qtr * 1024 : (qtr + 1) * 1024], o16[:])

        # ---------- interleaved emission (PE and DVE queues in-order) ----
        # per-tile one-hots let tile-t gathers start right after chain(t);
        # early units interleave into the score phase to fill DVE-pacing
        # stalls, alternating tiles so q8 quarters stream in at half rate
        emit_score(0); emit_score(1); emit_score(2); emit_score(3)
        emit_tp(0); emit_oh(0)
        emit_gunit(0, 0, 0)
        emit_score(4)
        emit_tp(1); emit_oh(1)
        emit_gunit(1, 0, 1)
        emit_score(5)
        emit_tp(2); emit_oh(2)
        emit_gunit(0, 1, 2)
        emit_score(6)
        emit_tp(3); emit_oh(3)
        emit_gunit(1, 1, 3)
        emit_score(7)
        emit_gunit(0, 2, 4)
        emit_tp(4); emit_oh(4)
        emit_gunit(1, 2, 5)
        emit_tp(5); emit_oh(5)
        emit_gunit(0, 3, 6)
        emit_tp(6); emit_oh(6)
        emit_gunit(1, 3, 7)
        emit_tp(7); emit_oh(7)
        u = 8
        for t in range(2, TT):
            for qtr in range(NQ):
                emit_gunit(t, qtr, u)
                u += 1

    nc.compile()
    return nc


def _consts():
    kiota = (79.0 - np.arange(128, dtype=np.float32) % KC).reshape(128, 1).astype(np.float16)
    ioneg = np.tile(
        15.0 - (np.arange(1024, dtype=np.float32) % KC), (128, 1)
    ).astype(np.float16)
    idb = np.eye(128, dtype=np.float16)
    return kiota, ioneg, idb


def _prep_inputs(x, centroids, weight, bias):
    """Host-side shard/layout prep, exact int8 fake-quant lut, fp8 split."""
    import ml_dtypes

    kiota, ioneg, idb = _consts()
    # block-diagonal centroids^T: bd[16j+s, g, 16j+k] = centroids[8g+j, k, s]
    bd = np.zeros((128, G, 128), np.float32)
    for g in range(G):
        for j in range(8):
            bd[16 * j : 16 * (j + 1), g, 16 * j : 16 * (j + 1)] = centroids[
                8 * g + j
            ].T
    nc2 = (-0.5 * (centroids.astype(np.float64) ** 2).sum(-1)).astype(
        np.float32
    ).reshape(1, C * KC)
    nc2h = nc2.astype(np.float16)
    nc2l = (nc2 - nc2h.astype(np.float32)).astype(np.float16)
    nc2hl = np.concatenate([nc2h, nc2l], axis=0)
    or2 = np.ones((2, 128), np.float16)

    # exact int8 fake-quant lut (float64, matching the oracle) + fp8 split
    lut = np.einsum(
        "cks,cso->cko", centroids.astype(np.float64), weight.astype(np.float64)
    )
    amax = np.abs(lut).max()
    scale = np.float32(amax / 127.0)
    q = np.clip(np.round(lut / (amax / 127.0)), -127.0, 127.0)
    qa16 = 16.0 * np.round(q / 16.0)   # multiples of 16, |.| <= 128
    qb = q - qa16                      # ints, |.| <= 8

    def pack(v):
        v = v.reshape(G, 8, KC, O)       # [g, j, k, o]
        v = v.transpose(1, 2, 0, 3)      # [j, k, g, o] -> p = 16j+k
        v = v.reshape(128, G, NQ, 1024)  # [p, g, qtr, oc]
        v = v.transpose(0, 2, 1, 3)      # [p, qtr, g, oc]
        return np.ascontiguousarray(v).astype(ml_dtypes.float8_e4m3)

    common = dict(
        q8a=pack(qa16), q8b=pack(qb), bdf=bd, nc2hl=nc2hl,
        or2=or2, kiota=kiota, ioneg=ioneg, idb=idb,
    )
    in_maps = []
    for i in range(NCORES):
        xs = x[i * NLOC : (i + 1) * NLOC, :]  # (1024, 1024)
        xf = np.ascontiguousarray(
            xs.T.reshape(G, 128, TT, 128).transpose(1, 2, 0, 3)
        )  # [p, t, g, n] float32
        m = dict(common)
        m.update(xf=xf)
        in_maps.append(m)
    return in_maps, scale, np.asarray(bias, np.float32)


def _assemble(per_core_outs, scale, bias32):
    out = np.concatenate(
        [np.asarray(o).astype(np.float32) for o in per_core_outs], axis=0
    )
    return out * scale + bias32[None, :]


def kernel(x, centroids, weight, inverse_temperature_logit, bias, **_):
    from concourse.bass_utils import run_bass_kernel_spmd

    x = np.asarray(x, np.float32)
    centroids = np.asarray(centroids, np.float32)
    weight = np.asarray(weight, np.float32)
    bias = np.asarray(bias, np.float32)

    if "nc" not in _CACHED:
        _CACHED["nc"] = build_nc()
    nc = _CACHED["nc"]

    in_maps, scale, bias32 = _prep_inputs(x, centroids, weight, bias)
    res = run_bass_kernel_spmd(nc, in_maps, core_ids=list(range(NCORES)))
    return _assemble(
        [res.results[i]["out"] for i in range(NCORES)], scale, bias32
    )
